# revision 50
# baseline (speedup 1.0000x reference)
"""Trainium2 Bass kernel for nn_CPWGenerator (B=16384, D=128, P=10, F=1024).

Data-parallel over batch across 8 NeuronCores (2048 rows/core). Per core:
  - feature-major 3-layer MLPs (control-point head + weight head)
  - softmax denominator cancels: out = num/den with raw e = exp(logits)
    (scale-invariant; reference's +1e-8 eps shifts results by <1.1e-7)
  - RATIO INTERPOLATION: out(t) = N(t)/D(t) is a ratio of Gaussian
    mixtures with sigma = 0.1 in t-units. Evaluate the ratio at S=128
    uniform sample points (unnormalized basis -- normalization cancels),
    then cubic-Lagrange interpolate to the F=1024 feature grid with
    [S,F] fp16 matmuls per 128-row batch chunk. Interp error ~1e-4;
    fp16 operand rounding ~5e-4 -- budget is 2e-2.
  - output written to HBM as fp16 (values are convex combos of tanh
    outputs, |out| <= 1), host converts to fp32: halves out-DMA bytes.
  - wl matmul uses duplicated W3w columns so exp() lands directly on a
    [20,NB] e_dup tile; the pairing matmul is folded into the sample
    basis (rows 2p/2p+1 carry 0.5*phi_p).
  - GPSIMD cannot touch PSUM on TRN2, so all PSUM evacuations run on
    ACT/DVE; GPSIMD handles SBUF-only work (ecp, fp32->fp16 converts
    behind a DVE uint64-bitcast PSUM copy that halves DVE column count).
Matmuls run as float32r (fp32 storage, 11-bit-mantissa operand rounding,
exact fp32 accumulation) at full PE rate; interp matmuls run fp16.
"""
import sys
if "/opt/trn_rl_repo" not in sys.path:
    sys.path.insert(0, "/opt/trn_rl_repo")

from contextlib import ExitStack

import numpy as np

import concourse.bacc as bacc
import concourse.mybir as mybir
import concourse.tile as tile
from concourse.bass_utils import run_bass_kernel_spmd

F32 = mybir.dt.float32
F32R = mybir.dt.float32r
F16 = mybir.dt.float16
U64 = mybir.dt.uint64
AF = mybir.ActivationFunctionType
ALU = mybir.AluOpType

# problem shapes (hardcoded per contest contract)
B, D, P, F = 16384, 128, 10, 1024
NCORES = 8
BC = B // NCORES          # rows per core = 2048
NB = 512                  # batch block
NBLK = BC // NB           # 4 blocks
S = 128                   # ratio sample count
EPS = 1e-8

# f32r const blob column offsets (layer-1 weights first: they ride a
# small early DMA that ungates the first matmuls)
_C_W1T = 0            # [128 x 128]
_C_WW1T = 128         # [128 x 64]
C_R1 = 192            # first-DMA split point
_C_W2T = 192          # [128 x 256]
_C_W3T = 448          # [128 x 40]  (W3Ta | W3Tb, 20 cols each)
_C_WW2T = 488         # [64  x 128]
_C_WW3D = 616         # [128 x 20]  w MLP final, columns duplicated per pair
_C_BTSP = 636         # [20  x S]   0.5 * phi_p(t_s) on rows 2p, 2p+1
C_R = 636 + S

# fp32 const blob columns (biases)
_C_B1 = 0
_C_B2A = 1
_C_B2B = 2
_C_B3 = 3
_C_WB1 = 4
_C_WB2 = 5
_C_WB3D = 6           # w MLP final bias duplicated per pair (20 rows)
C_F = 7


def round_f32r(x: np.ndarray) -> np.ndarray:
    """fp32 -> fp32r rounding (keep 11 explicit mantissa bits, RNE).
    Matches TRN2 hardware exactly (validated on device)."""
    u = np.ascontiguousarray(x, dtype=np.float32).view(np.uint32)
    keep = np.uint32(0xFFFFF000)
    half = np.uint32(0x800)
    lsb = (u >> np.uint32(12)) & np.uint32(1)
    r = (u + half - np.uint32(1) + lsb) & keep
    return r.view(np.float32)


def sample_basis() -> np.ndarray:
    """[20, S]: rows 2p and 2p+1 hold 0.5*phi_p(t_s), unnormalized
    Gaussian basis at the S uniform sample points (normalization cancels
    in the num/den ratio; the 0.5 pair split computes cp_mean)."""
    ts = np.arange(S, dtype=np.float64) / (S - 1)
    centers = np.arange(P, dtype=np.float64) / (P - 1)
    sigma = 1.0 / P
    phi = np.exp(-((ts[None, :] - centers[:, None]) ** 2)
                 / (2.0 * sigma * sigma))          # [P, S]
    bt = np.zeros((2 * P, S), np.float64)
    bt[0::2] = 0.5 * phi
    bt[1::2] = 0.5 * phi
    return bt.astype(np.float32)


def interp_matrix() -> np.ndarray:
    """[S, F] cubic-Lagrange interpolation matrix from the S uniform
    sample grid to the F uniform feature grid (both span [0,1])."""
    ts = np.arange(S, dtype=np.float64) / (S - 1)
    M = np.zeros((S, F), np.float64)
    for f in range(F):
        tf = f / (F - 1)
        j = int(np.floor(tf * (S - 1)))
        j0 = min(max(j - 1, 0), S - 4)
        xs = ts[j0:j0 + 4]
        for a in range(4):
            L = 1.0
            for b_ in range(4):
                if a != b_:
                    L *= (tf - xs[b_]) / (xs[a] - xs[b_])
            M[j0 + a, f] = L
    return M.astype(np.float16)


# engine assignment config (sweepable)
CFG = {
    # engines for the relu/copy evacuations (PSUM readers: act/dve only)
    "xt": "dve", "h1": "dve", "h2a": "act", "h2b": "act",
    "g1": "dve", "g2": "act",
    "ecp": "pool",
    # per-half interp evac modes, cycled: "act"/"dve" = direct fp16 copy;
    # "u64" = DVE uint64 psum copy (half cols) + Pool sbuf fp16 convert
    "evac": ["dve", "act", "act", "dve", "act", "dve", "dve", "act"],
    "order": None,
    "skew": None,
    "waves": [0, 3, 6, 7.8],
    # block 0: early-chain evacs on ACT so its dense stream starts early
    "b0": {"xt": "act", "g1": "act"},
    "evac3": None,
}


def build_program():
    nc = bacc.Bacc()
    x_in = nc.declare_dram_parameter("x", [BC, D], F32R, isOutput=False)
    wr_in = nc.declare_dram_parameter("wr", [128, C_R], F32R, isOutput=False)
    wf_in = nc.declare_dram_parameter("wf", [128, C_F], F32, isOutput=False)
    im_in = nc.declare_dram_parameter("im", [S, F], F16, isOutput=False)
    out = nc.declare_dram_parameter("out", [BC, F], F16, isOutput=True)

    with tile.TileContext(nc) as tc, ExitStack() as ctx:
        cpool = ctx.enter_context(tc.tile_pool(name="const", bufs=1))
        _wnames = ["xt", "h1", "h2a", "h2b", "cp", "g1", "g2", "eec",
                   "rs", "outs"]
        vp = {n: ctx.enter_context(tc.tile_pool(name=n, bufs=2))
              for n in _wnames}
        opool = ctx.enter_context(tc.tile_pool(name="outp", bufs=8))
        spool = ctx.enter_context(tc.tile_pool(name="stg", bufs=4))
        pp2 = [ctx.enter_context(tc.tile_pool(name=f"psum{i}", bufs=2,
                                              space="PSUM"))
               for i in range(2)]
        qpool = ctx.enter_context(tc.tile_pool(name="psumo", bufs=4,
                                               space="PSUM"))

        identt = cpool.tile([128, 128], F32R)
        wr = cpool.tile([128, C_R], F32R)
        wf = cpool.tile([128, C_F], F32)
        im = cpool.tile([S, F], F16)
        xb = [cpool.tile([128, NB], F32R, name=f"xb{i}")
              for i in range(NBLK)]

        def x_dma(blk):
            nc.sync.dma_start(
                xb[blk][:].rearrange("p (c d) -> p c d", c=NB // 128),
                x_in[blk * NB:(blk + 1) * NB, :].rearrange(
                    "(c p) d -> p c d", p=128),
            )

        # identity built on-device (no DMA dependency for the transposes)
        nc.gpsimd.memset(identt[:].bitcast(F32), 1.0)
        nc.gpsimd.affine_select(identt[:], identt[:], [[1, 128]],
                                mybir.AluOpType.is_equal, 0.0,
                                base=0, channel_multiplier=-1)
        # dummy table-func activation: hoists the 1.28us ACT table load
        # off the critical chain (it otherwise lands right before exp(0))
        scr = cpool.tile([1, 1], F32)
        nc.gpsimd.memset(scr[:], 0.0)
        nc.scalar.activation(scr[:], scr[:], AF.Relu)

        # in-DMA order tuned for pipeline fill: x block 0, layer-1
        # weights, biases, remaining weights, the rest
        if CFG.get("splitx0"):
            h = NB // 2
            nc.sync.dma_start(
                xb[0][:, 0:h].rearrange("p (c d) -> p c d", c=h // 128),
                x_in[0:h, :].rearrange("(c p) d -> p c d", p=128))
            nc.sync.dma_start(
                xb[0][:, h:NB].rearrange("p (c d) -> p c d", c=h // 128),
                x_in[h:NB, :].rearrange("(c p) d -> p c d", p=128))
        else:
            x_dma(0)
        nc.sync.dma_start(wr[:, 0:C_R1], wr_in[:, 0:C_R1])
        nc.sync.dma_start(wf[:], wf_in[:])
        nc.sync.dma_start(wr[:, C_R1:C_R], wr_in[:, C_R1:C_R])
        x_dma(1)
        x_dma(2)
        nc.sync.dma_start(im[:], im_in[:])
        for blk in range(3, NBLK):
            x_dma(blk)

        ident = identt[:]
        ENG = {"act": nc.scalar, "dve": nc.vector, "pool": nc.gpsimd}

        def mm(out_ap, lhsT, rhs, start=True, stop=True):
            nc.tensor.matmul(out_ap, lhsT, rhs, start=start, stop=stop)

        # psum -> sbuf evacuation with relu+bias: act/dve direct, or
        # "u64p" = DVE uint64 raw copy (half cols) + Pool relu (SBUF-only)
        def evac_relu(name, dst, src, bias_col, rows=128, blk=None):
            eng = (CFG.get(f"b{blk}") or {}).get(name) or CFG[name]
            if eng == "act":
                nc.scalar.activation(dst[:], src, AF.Relu,
                                     bias=wf[0:rows, bias_col:bias_col + 1])
            elif eng == "u64p":
                stg = spool.tile([rows, NB], F32, name=f"stg_{name}")
                nc.vector.tensor_copy(stg[:].bitcast(U64), src.bitcast(U64))
                nc.gpsimd.tensor_scalar(
                    dst[:], stg[:], wf[0:rows, bias_col:bias_col + 1],
                    0.0, ALU.add, ALU.max)
            else:
                ENG[eng].tensor_scalar(
                    dst[:], src, wf[0:rows, bias_col:bias_col + 1],
                    0.0, ALU.add, ALU.max)

        outs_t = [None] * NBLK
        state = [dict() for _ in range(NBLK)]

        def front_atoms(blk):
            """Staged atoms: transpose + MLPs + ratio samples -> outs."""
            ppool = pp2[blk % 2]
            st = state[blk]

            def a_xt():
                xtp = ppool.tile([128, NB], F32R, tag="ps")
                for c in range(NB // 128):
                    nc.tensor.matmul(
                        xtp[:, 128 * c:128 * (c + 1)],
                        xb[blk][:, 128 * c:128 * (c + 1)],
                        ident,
                        is_transpose=True,
                        start=(c == 0),
                        stop=(c == NB // 128 - 1),
                    )
                xt = vp["xt"].tile([128, NB], F32R)
                xte = (CFG.get(f"b{blk}") or {}).get("xt") or CFG["xt"]
                if xte == "act":
                    nc.scalar.activation(xt[:], xtp[:].bitcast(F32), AF.Copy)
                elif xte == "u64p":
                    stg = spool.tile([128, NB], F32, name="stg_xt")
                    nc.vector.tensor_copy(stg[:].bitcast(U64),
                                          xtp[:].bitcast(U64))
                    nc.gpsimd.tensor_copy(xt[:], stg[:])
                else:
                    nc.vector.tensor_copy(xt[:], xtp[:].bitcast(F32))
                st["xt"] = xt

            def a_g1():
                g1p = ppool.tile([64, NB], F32, tag="ps")
                mm(g1p[:], wr[:, _C_WW1T:_C_WW1T + 64], st["xt"][:])
                g1 = vp["g1"].tile([64, NB], F32R)
                evac_relu("g1", g1, g1p[:], _C_WB1, rows=64, blk=blk)
                st["g1"] = g1

            def a_g2():
                g2p = ppool.tile([128, NB], F32, tag="ps")
                mm(g2p[:], wr[0:64, _C_WW2T:_C_WW2T + 128], st["g1"][:])
                g2 = vp["g2"].tile([128, NB], F32R)
                evac_relu("g2", g2, g2p[:], _C_WB2, blk=blk)
                st["g2"] = g2

            def a_exp():
                wlp = ppool.tile([20, NB], F32, tag="ps")
                mm(wlp[:], wr[:, _C_WW3D:_C_WW3D + 20], st["g2"][:])
                eec = vp["eec"].tile([20, 2 * NB], F32R)
                nc.scalar.activation(eec[:, 0:NB], wlp[:], AF.Exp,
                                     bias=wf[0:20, _C_WB3D:_C_WB3D + 1])
                st["eec"] = eec

            def a_h1():
                h1p = ppool.tile([128, NB], F32, tag="ps")
                mm(h1p[:], wr[:, _C_W1T:_C_W1T + 128], st["xt"][:])
                h1 = vp["h1"].tile([128, NB], F32R)
                evac_relu("h1", h1, h1p[:], _C_B1, blk=blk)
                st["h1"] = h1

            def a_h2a():
                h2pa = ppool.tile([128, NB], F32, tag="ps")
                mm(h2pa[:], wr[:, _C_W2T:_C_W2T + 128], st["h1"][:])
                h2a = vp["h2a"].tile([128, NB], F32R)
                evac_relu("h2a", h2a, h2pa[:], _C_B2A, blk=blk)
                st["h2a"] = h2a

            def a_h2b():
                h2pb = ppool.tile([128, NB], F32, tag="ps")
                mm(h2pb[:], wr[:, _C_W2T + 128:_C_W2T + 256], st["h1"][:])
                h2b = vp["h2b"].tile([128, NB], F32R)
                evac_relu("h2b", h2b, h2pb[:], _C_B2B, blk=blk)
                st["h2b"] = h2b

            def a_tanh():
                cpp = ppool.tile([20, NB], F32, tag="ps")
                mm(cpp[:], wr[:, _C_W3T:_C_W3T + 20], st["h2a"][:],
                   stop=False)
                mm(cpp[:], wr[:, _C_W3T + 20:_C_W3T + 40], st["h2b"][:],
                   start=False, stop=True)
                cp = vp["cp"].tile([20, NB], F32R)
                nc.scalar.activation(cp[:], cpp[:], AF.Tanh,
                                     bias=wf[0:20, _C_B3:_C_B3 + 1])
                st["cp"] = cp

            def a_ecp():
                eec = st["eec"]
                ENG[CFG["ecp"]].tensor_mul(
                    eec[:, NB:2 * NB], st["cp"][:].bitcast(F32),
                    eec[:, 0:NB].bitcast(F32))

            def a_ratio():
                eec = st["eec"]
                den = qpool.tile([S, NB], F32, tag="out")
                mm(den[:], wr[0:20, _C_BTSP:_C_BTSP + S], eec[:, 0:NB])
                num = qpool.tile([S, NB], F32, tag="out")
                mm(num[:], wr[0:20, _C_BTSP:_C_BTSP + S], eec[:, NB:2 * NB])
                rs = vp["rs"].tile([S, NB], F32)
                nc.vector.reciprocal_approx_fast(out=rs[:], in_=den[:])
                outs = vp["outs"].tile([S, NB], F16)
                nc.vector.tensor_mul(outs[:], num[:], rs[:])
                outs_t[blk] = outs

            return [(0.0, a_xt), (1.0, a_g1), (2.0, a_g2), (3.0, a_exp),
                    (1.5, a_h1), (2.5, a_h2a), (2.7, a_h2b), (3.5, a_tanh),
                    (4.0, a_ecp), (4.6, a_ratio)]

        evac_rr = [0]

        def back_atoms(blk):
            """One atom per 128-row chunk: 2 interp mms + evacs + DMA."""
            atoms = []
            for j in range(NB // 128):
                def a_chunk(j=j):
                    outs = outs_t[blk]
                    obuf = opool.tile([128, F], F16)
                    for h in range(F // 512):
                        fsl = slice(512 * h, 512 * (h + 1))
                        obp = qpool.tile([128, 512], F32, tag="out")
                        mm(obp[:], outs[:, 128 * j:128 * (j + 1)],
                           im[:, fsl])
                        ev = CFG.get(f"evac{blk}") or CFG["evac"]
                        mode = ev[evac_rr[0] % len(ev)]
                        evac_rr[0] += 1
                        if mode == "act":
                            nc.scalar.copy(obuf[:, fsl], obp[:])
                        else:
                            nc.vector.tensor_copy(obuf[:, fsl], obp[:])
                    r0 = blk * NB + j * 128
                    if CFG.get("taildma") and blk == NBLK - 1 and \
                            j == NB // 128 - 1:
                        # final chunk: two half-DMAs so the very last
                        # transfer behind the last evac is half-size
                        nc.sync.dma_start(out[r0:r0 + 128, 0:512],
                                          obuf[:, 0:512])
                        nc.sync.dma_start(out[r0:r0 + 128, 512:F],
                                          obuf[:, 512:F])
                    else:
                        nc.sync.dma_start(out[r0:r0 + 128, :], obuf[:])
                atoms.append((CFG.get("bstage", 5.6)
                              + CFG.get("bgap", 0.5) * j, a_chunk))
            return atoms

        waves = CFG.get("waves")
        skew = CFG.get("skew")
        if waves is not None:
            allatoms = []
            for k in range(NBLK):
                for s, fn in front_atoms(k) + back_atoms(k):
                    allatoms.append((s + waves[k], k, s, fn))
            allatoms.sort(key=lambda t: (t[0], t[1], t[2]))
            for _, _, _, fn in allatoms:
                fn()
            skew = "done"
        if skew == "done":
            pass
        elif skew is None:
            # block-sequential emission per CFG order string
            fronts = [front_atoms(k) for k in range(NBLK)]
            backs = [back_atoms(k) for k in range(NBLK)]
            fi, bi = 0, 0
            order = CFG["order"] or ("AA" + "BA" * (NBLK - 2) + "BB")
            for ch in order:
                if ch == "A":
                    for _, fn in fronts[fi]:
                        fn()
                    fi += 1
                else:
                    for _, fn in backs[bi]:
                        fn()
                    bi += 1
            assert fi == NBLK and bi == NBLK
        else:
            # wavefront emission: priority = stage + blk * skew
            allatoms = []
            for k in range(NBLK):
                for s, fn in front_atoms(k) + back_atoms(k):
                    allatoms.append((s + k * skew, k, s, fn))
            allatoms.sort(key=lambda t: (t[0], t[1], t[2]))
            for _, _, _, fn in allatoms:
                fn()

    nc.compile()
    return nc


def host_consts(cp_w1, cp_b1, cp_w2, cp_b2, cp_w3, cp_b3,
                w_w1, w_b1, w_w2, w_b2, w_w3, w_b3):
    wr = np.zeros((128, C_R), np.float32)
    wr[:, _C_W1T:_C_W1T + 128] = cp_w1.T       # [128,128]
    wr[:, _C_W2T:_C_W2T + 256] = cp_w2.T       # [128,256]
    w3t = cp_w3.T                              # [256,20]
    wr[:, _C_W3T:_C_W3T + 20] = w3t[0:128]
    wr[:, _C_W3T + 20:_C_W3T + 40] = w3t[128:256]
    wr[:, _C_WW1T:_C_WW1T + 64] = w_w1.T       # [128,64]
    wr[0:64, _C_WW2T:_C_WW2T + 128] = w_w2.T   # [64,128]
    w3w = w_w3.T                               # [128,10]
    wr[:, _C_WW3D + 0:_C_WW3D + 20:2] = w3w
    wr[:, _C_WW3D + 1:_C_WW3D + 20:2] = w3w
    wr[0:20, _C_BTSP:_C_BTSP + S] = sample_basis()
    wr = round_f32r(wr)

    wf = np.zeros((128, C_F), np.float32)
    wf[:, _C_B1] = cp_b1
    wf[:, _C_B2A] = cp_b2[0:128]
    wf[:, _C_B2B] = cp_b2[128:256]
    wf[0:20, _C_B3] = cp_b3
    wf[0:64, _C_WB1] = w_b1
    wf[:, _C_WB2] = w_b2
    wf[0:20, _C_WB3D:_C_WB3D + 1] = np.repeat(w_b3, 2)[:, None]
    im = interp_matrix()
    return wr, wf, im


_NC_CACHE = None


def get_program():
    global _NC_CACHE
    if _NC_CACHE is None:
        _NC_CACHE = build_program()
    return _NC_CACHE


def kernel(x, cp_w1, cp_b1, cp_w2, cp_b2, cp_w3, cp_b3,
           w_w1, w_b1, w_w2, w_b2, w_w3, w_b3, _return_raw=False):
    x = np.asarray(x, np.float32)
    wr, wf, im = host_consts(
        np.asarray(cp_w1, np.float32), np.asarray(cp_b1, np.float32),
        np.asarray(cp_w2, np.float32), np.asarray(cp_b2, np.float32),
        np.asarray(cp_w3, np.float32), np.asarray(cp_b3, np.float32),
        np.asarray(w_w1, np.float32), np.asarray(w_b1, np.float32),
        np.asarray(w_w2, np.float32), np.asarray(w_b2, np.float32),
        np.asarray(w_w3, np.float32), np.asarray(w_b3, np.float32))

    nc = get_program()
    in_maps = [
        {"x": np.ascontiguousarray(x[i * BC:(i + 1) * BC]),
         "wr": wr, "wf": wf, "im": im}
        for i in range(NCORES)
    ]
    res = run_bass_kernel_spmd(nc, in_maps, list(range(NCORES)))
    outs = [res.results[i]["out"] for i in range(NCORES)]
    full = np.concatenate(outs, axis=0).astype(np.float32)
    if _return_raw:
        return full, res
    return full


# revision 52
# speedup vs baseline: 1.0563x; 1.0563x over previous
"""Trainium2 Bass kernel for nn_CPWGenerator (B=16384, D=128, P=10, F=1024).

Data-parallel over batch across 8 NeuronCores (2048 rows/core). Per core:
  - feature-major 3-layer MLPs (control-point head + weight head)
  - softmax denominator cancels: out = num/den with raw e = exp(logits)
    (scale-invariant; reference's +1e-8 eps shifts results by <1.1e-7)
  - RATIO INTERPOLATION: out(t) = N(t)/D(t) is a ratio of Gaussian
    mixtures with sigma = 0.1 in t-units. Evaluate the ratio at S=128
    uniform sample points (unnormalized basis -- normalization cancels),
    then cubic-Lagrange interpolate to the F=1024 feature grid with
    [S,F] fp16 matmuls per 128-row batch chunk. Interp error ~1e-4;
    fp16 operand rounding ~5e-4 -- budget is 2e-2.
  - output written to HBM as fp16 (values are convex combos of tanh
    outputs, |out| <= 1), host converts to fp32: halves out-DMA bytes.
  - wl matmul uses duplicated W3w columns so exp() lands directly on a
    [20,NB] e_dup tile; the pairing matmul is folded into the sample
    basis (rows 2p/2p+1 carry 0.5*phi_p).
  - GPSIMD cannot touch PSUM on TRN2, so all PSUM evacuations run on
    ACT/DVE; GPSIMD handles SBUF-only work (ecp, fp32->fp16 converts
    behind a DVE uint64-bitcast PSUM copy that halves DVE column count).
Matmuls run as float32r (fp32 storage, 11-bit-mantissa operand rounding,
exact fp32 accumulation) at full PE rate; interp matmuls run fp16.
"""
import sys
if "/opt/trn_rl_repo" not in sys.path:
    sys.path.insert(0, "/opt/trn_rl_repo")

from contextlib import ExitStack

import numpy as np

import concourse.bacc as bacc
import concourse.mybir as mybir
import concourse.tile as tile
from concourse.bass_utils import run_bass_kernel_spmd

F32 = mybir.dt.float32
F32R = mybir.dt.float32r
F16 = mybir.dt.float16
U64 = mybir.dt.uint64
AF = mybir.ActivationFunctionType
ALU = mybir.AluOpType

# problem shapes (hardcoded per contest contract)
B, D, P, F = 16384, 128, 10, 1024
NCORES = 8
BC = B // NCORES          # rows per core = 2048
NB = 512                  # batch block
NBLK = BC // NB           # 4 blocks
S = 128                   # ratio sample count
EPS = 1e-8

# f32r const blob column offsets (layer-1 weights first: they ride a
# small early DMA that ungates the first matmuls)
_C_W1T = 0            # [128 x 128]
_C_WW1T = 128         # [128 x 64]
C_R1 = 192            # first-DMA split point
_C_W2T = 192          # [128 x 256]
_C_W3T = 448          # [128 x 40]  (W3Ta | W3Tb, 20 cols each)
_C_WW2T = 488         # [64  x 128]
_C_WW3D = 616         # [128 x 20]  w MLP final, columns duplicated per pair
_C_BTSP = 636         # [20  x S]   0.5 * phi_p(t_s) on rows 2p, 2p+1
C_R = 636 + S

# fp32 const blob columns (biases)
_C_B1 = 0
_C_B2A = 1
_C_B2B = 2
_C_B3 = 3
_C_WB1 = 4
_C_WB2 = 5
_C_WB3D = 6           # w MLP final bias duplicated per pair (20 rows)
C_F = 7


def round_f32r(x: np.ndarray) -> np.ndarray:
    """fp32 -> fp32r rounding (keep 11 explicit mantissa bits, RNE).
    Matches TRN2 hardware exactly (validated on device)."""
    u = np.ascontiguousarray(x, dtype=np.float32).view(np.uint32)
    keep = np.uint32(0xFFFFF000)
    half = np.uint32(0x800)
    lsb = (u >> np.uint32(12)) & np.uint32(1)
    r = (u + half - np.uint32(1) + lsb) & keep
    return r.view(np.float32)


def sample_basis() -> np.ndarray:
    """[20, S]: rows 2p and 2p+1 hold 0.5*phi_p(t_s), unnormalized
    Gaussian basis at the S uniform sample points (normalization cancels
    in the num/den ratio; the 0.5 pair split computes cp_mean)."""
    ts = np.arange(S, dtype=np.float64) / (S - 1)
    centers = np.arange(P, dtype=np.float64) / (P - 1)
    sigma = 1.0 / P
    phi = np.exp(-((ts[None, :] - centers[:, None]) ** 2)
                 / (2.0 * sigma * sigma))          # [P, S]
    bt = np.zeros((2 * P, S), np.float64)
    bt[0::2] = 0.5 * phi
    bt[1::2] = 0.5 * phi
    return bt.astype(np.float32)


def interp_matrix() -> np.ndarray:
    """[S, F] cubic-Lagrange interpolation matrix from the S uniform
    sample grid to the F uniform feature grid (both span [0,1])."""
    ts = np.arange(S, dtype=np.float64) / (S - 1)
    M = np.zeros((S, F), np.float64)
    for f in range(F):
        tf = f / (F - 1)
        j = int(np.floor(tf * (S - 1)))
        j0 = min(max(j - 1, 0), S - 4)
        xs = ts[j0:j0 + 4]
        for a in range(4):
            L = 1.0
            for b_ in range(4):
                if a != b_:
                    L *= (tf - xs[b_]) / (xs[a] - xs[b_])
            M[j0 + a, f] = L
    return M.astype(np.float16)


# engine assignment config (sweepable)
CFG = {
    # engines for the relu/copy evacuations (PSUM readers: act/dve only)
    "xt": "dve", "h1": "dve", "h2a": "act", "h2b": "act",
    "g1": "dve", "g2": "dve",
    "ecp": "pool",
    # per-half interp evac modes, cycled: "act"/"dve" = direct fp16 copy;
    # "u64" = DVE uint64 psum copy (half cols) + Pool sbuf fp16 convert
    "evac": ["dve", "act", "act", "dve", "act", "dve", "act", "act"],
    "order": None,
    "skew": None,
    "waves": [0, 2.2, 4.5, 6.2],
    # block 0: early-chain evac on ACT so its dense stream starts early
    "b0": {"g1": "act"},
    "evac3": None,
}


def build_program():
    nc = bacc.Bacc()
    x_in = nc.declare_dram_parameter("x", [D, BC], F32R, isOutput=False)
    wr_in = nc.declare_dram_parameter("wr", [128, C_R], F32R, isOutput=False)
    wf_in = nc.declare_dram_parameter("wf", [128, C_F], F32, isOutput=False)
    im_in = nc.declare_dram_parameter("im", [S, F], F16, isOutput=False)
    out = nc.declare_dram_parameter("out", [BC, F], F16, isOutput=True)

    with tile.TileContext(nc) as tc, ExitStack() as ctx:
        cpool = ctx.enter_context(tc.tile_pool(name="const", bufs=1))
        _wnames = ["xt", "h1", "h2a", "h2b", "cp", "g1", "g2", "eec",
                   "rs", "outs"]
        vp = {n: ctx.enter_context(tc.tile_pool(name=n, bufs=2))
              for n in _wnames}
        opool = ctx.enter_context(tc.tile_pool(name="outp", bufs=8))
        spool = ctx.enter_context(tc.tile_pool(name="stg", bufs=4))
        pp2 = [ctx.enter_context(tc.tile_pool(name=f"psum{i}", bufs=2,
                                              space="PSUM"))
               for i in range(2)]
        qpool = ctx.enter_context(tc.tile_pool(name="psumo", bufs=4,
                                               space="PSUM"))

        wr = cpool.tile([128, C_R], F32R)
        wf = cpool.tile([128, C_F], F32)
        im = cpool.tile([S, F], F16)
        xb = [cpool.tile([128, NB], F32R, name=f"xb{i}")
              for i in range(NBLK)]

        def x_dma(blk):
            nc.sync.dma_start(xb[blk][:],
                              x_in[:, blk * NB:(blk + 1) * NB])

        # dummy table-func activation: hoists the 1.28us ACT table load
        # off the critical chain (it otherwise lands right before exp(0))
        scr = cpool.tile([1, 1], F32)
        nc.gpsimd.memset(scr[:], 0.0)
        nc.scalar.activation(scr[:], scr[:], AF.Relu)

        # in-DMA order tuned for pipeline fill: x block 0, layer-1
        # weights, biases, remaining weights, the rest
        if CFG.get("splitx0"):
            h = NB // 2
            nc.sync.dma_start(
                xb[0][:, 0:h].rearrange("p (c d) -> p c d", c=h // 128),
                x_in[0:h, :].rearrange("(c p) d -> p c d", p=128))
            nc.sync.dma_start(
                xb[0][:, h:NB].rearrange("p (c d) -> p c d", c=h // 128),
                x_in[h:NB, :].rearrange("(c p) d -> p c d", p=128))
        else:
            x_dma(0)
        nc.sync.dma_start(wr[:, 0:C_R1], wr_in[:, 0:C_R1])
        nc.sync.dma_start(wf[:], wf_in[:])
        nc.sync.dma_start(wr[:, C_R1:C_R], wr_in[:, C_R1:C_R])
        x_dma(1)
        x_dma(2)
        nc.sync.dma_start(im[:], im_in[:])
        for blk in range(3, NBLK):
            x_dma(blk)

        ENG = {"act": nc.scalar, "dve": nc.vector, "pool": nc.gpsimd}

        def mm(out_ap, lhsT, rhs, start=True, stop=True):
            nc.tensor.matmul(out_ap, lhsT, rhs, start=start, stop=stop)

        # psum -> sbuf evacuation with relu+bias: act/dve direct, or
        # "u64p" = DVE uint64 raw copy (half cols) + Pool relu (SBUF-only)
        def evac_relu(name, dst, src, bias_col, rows=128, blk=None):
            eng = (CFG.get(f"b{blk}") or {}).get(name) or CFG[name]
            if eng == "act":
                nc.scalar.activation(dst[:], src, AF.Relu,
                                     bias=wf[0:rows, bias_col:bias_col + 1])
            elif eng == "u64p":
                stg = spool.tile([rows, NB], F32, name=f"stg_{name}")
                nc.vector.tensor_copy(stg[:].bitcast(U64), src.bitcast(U64))
                nc.gpsimd.tensor_scalar(
                    dst[:], stg[:], wf[0:rows, bias_col:bias_col + 1],
                    0.0, ALU.add, ALU.max)
            else:
                ENG[eng].tensor_scalar(
                    dst[:], src, wf[0:rows, bias_col:bias_col + 1],
                    0.0, ALU.add, ALU.max)

        outs_t = [None] * NBLK
        state = [dict() for _ in range(NBLK)]

        def front_atoms(blk):
            """Staged atoms: transpose + MLPs + ratio samples -> outs."""
            ppool = pp2[blk % 2]
            st = state[blk]

            def a_xt():
                # x arrives pre-transposed from the host: xb IS xT
                st["xt"] = xb[blk]

            def a_g1():
                g1p = ppool.tile([64, NB], F32, tag="ps")
                mm(g1p[:], wr[:, _C_WW1T:_C_WW1T + 64], st["xt"][:])
                g1 = vp["g1"].tile([64, NB], F32R)
                evac_relu("g1", g1, g1p[:], _C_WB1, rows=64, blk=blk)
                st["g1"] = g1

            def a_g2():
                g2p = ppool.tile([128, NB], F32, tag="ps")
                mm(g2p[:], wr[0:64, _C_WW2T:_C_WW2T + 128], st["g1"][:])
                g2 = vp["g2"].tile([128, NB], F32R)
                evac_relu("g2", g2, g2p[:], _C_WB2, blk=blk)
                st["g2"] = g2

            def a_exp():
                wlp = ppool.tile([20, NB], F32, tag="ps")
                mm(wlp[:], wr[:, _C_WW3D:_C_WW3D + 20], st["g2"][:])
                eec = vp["eec"].tile([20, 2 * NB], F32R)
                nc.scalar.activation(eec[:, 0:NB], wlp[:], AF.Exp,
                                     bias=wf[0:20, _C_WB3D:_C_WB3D + 1])
                st["eec"] = eec

            def a_h1():
                h1p = ppool.tile([128, NB], F32, tag="ps")
                mm(h1p[:], wr[:, _C_W1T:_C_W1T + 128], st["xt"][:])
                h1 = vp["h1"].tile([128, NB], F32R)
                evac_relu("h1", h1, h1p[:], _C_B1, blk=blk)
                st["h1"] = h1

            def a_h2a():
                h2pa = ppool.tile([128, NB], F32, tag="ps")
                mm(h2pa[:], wr[:, _C_W2T:_C_W2T + 128], st["h1"][:])
                h2a = vp["h2a"].tile([128, NB], F32R)
                evac_relu("h2a", h2a, h2pa[:], _C_B2A, blk=blk)
                st["h2a"] = h2a

            def a_h2b():
                h2pb = ppool.tile([128, NB], F32, tag="ps")
                mm(h2pb[:], wr[:, _C_W2T + 128:_C_W2T + 256], st["h1"][:])
                h2b = vp["h2b"].tile([128, NB], F32R)
                evac_relu("h2b", h2b, h2pb[:], _C_B2B, blk=blk)
                st["h2b"] = h2b

            def a_tanh():
                cpp = ppool.tile([20, NB], F32, tag="ps")
                mm(cpp[:], wr[:, _C_W3T:_C_W3T + 20], st["h2a"][:],
                   stop=False)
                mm(cpp[:], wr[:, _C_W3T + 20:_C_W3T + 40], st["h2b"][:],
                   start=False, stop=True)
                cp = vp["cp"].tile([20, NB], F32R)
                nc.scalar.activation(cp[:], cpp[:], AF.Tanh,
                                     bias=wf[0:20, _C_B3:_C_B3 + 1])
                st["cp"] = cp

            def a_ecp():
                eec = st["eec"]
                ENG[CFG["ecp"]].tensor_mul(
                    eec[:, NB:2 * NB], st["cp"][:].bitcast(F32),
                    eec[:, 0:NB].bitcast(F32))

            def a_ratio():
                eec = st["eec"]
                den = qpool.tile([S, NB], F32, tag="out")
                mm(den[:], wr[0:20, _C_BTSP:_C_BTSP + S], eec[:, 0:NB])
                num = qpool.tile([S, NB], F32, tag="out")
                mm(num[:], wr[0:20, _C_BTSP:_C_BTSP + S], eec[:, NB:2 * NB])
                rs = vp["rs"].tile([S, NB], F32)
                nc.vector.reciprocal_approx_fast(out=rs[:], in_=den[:])
                outs = vp["outs"].tile([S, NB], F16)
                nc.vector.tensor_mul(outs[:], num[:], rs[:])
                outs_t[blk] = outs

            return [(0.0, a_xt), (1.0, a_g1), (2.0, a_g2), (3.0, a_exp),
                    (1.5, a_h1), (2.5, a_h2a), (2.7, a_h2b), (3.5, a_tanh),
                    (4.0, a_ecp), (4.6, a_ratio)]

        evac_rr = [0]

        def back_atoms(blk):
            """One atom per 128-row chunk: 2 interp mms + evacs + DMA."""
            atoms = []
            for j in range(NB // 128):
                def a_chunk(j=j):
                    outs = outs_t[blk]
                    obuf = opool.tile([128, F], F16)
                    for h in range(F // 512):
                        fsl = slice(512 * h, 512 * (h + 1))
                        obp = qpool.tile([128, 512], F32, tag="out")
                        mm(obp[:], outs[:, 128 * j:128 * (j + 1)],
                           im[:, fsl])
                        ev = CFG.get(f"evac{blk}") or CFG["evac"]
                        mode = ev[evac_rr[0] % len(ev)]
                        evac_rr[0] += 1
                        if mode == "act":
                            nc.scalar.copy(obuf[:, fsl], obp[:])
                        else:
                            nc.vector.tensor_copy(obuf[:, fsl], obp[:])
                    r0 = blk * NB + j * 128
                    if CFG.get("taildma") and blk == NBLK - 1 and \
                            j == NB // 128 - 1:
                        # final chunk: two half-DMAs so the very last
                        # transfer behind the last evac is half-size
                        nc.sync.dma_start(out[r0:r0 + 128, 0:512],
                                          obuf[:, 0:512])
                        nc.sync.dma_start(out[r0:r0 + 128, 512:F],
                                          obuf[:, 512:F])
                    else:
                        nc.sync.dma_start(out[r0:r0 + 128, :], obuf[:])
                atoms.append((CFG.get("bstage", 5.6)
                              + CFG.get("bgap", 0.5) * j, a_chunk))
            return atoms

        waves = CFG.get("waves")
        skew = CFG.get("skew")
        if waves is not None:
            allatoms = []
            for k in range(NBLK):
                for s, fn in front_atoms(k) + back_atoms(k):
                    allatoms.append((s + waves[k], k, s, fn))
            allatoms.sort(key=lambda t: (t[0], t[1], t[2]))
            for _, _, _, fn in allatoms:
                fn()
            skew = "done"
        if skew == "done":
            pass
        elif skew is None:
            # block-sequential emission per CFG order string
            fronts = [front_atoms(k) for k in range(NBLK)]
            backs = [back_atoms(k) for k in range(NBLK)]
            fi, bi = 0, 0
            order = CFG["order"] or ("AA" + "BA" * (NBLK - 2) + "BB")
            for ch in order:
                if ch == "A":
                    for _, fn in fronts[fi]:
                        fn()
                    fi += 1
                else:
                    for _, fn in backs[bi]:
                        fn()
                    bi += 1
            assert fi == NBLK and bi == NBLK
        else:
            # wavefront emission: priority = stage + blk * skew
            allatoms = []
            for k in range(NBLK):
                for s, fn in front_atoms(k) + back_atoms(k):
                    allatoms.append((s + k * skew, k, s, fn))
            allatoms.sort(key=lambda t: (t[0], t[1], t[2]))
            for _, _, _, fn in allatoms:
                fn()

    nc.compile()
    return nc


def host_consts(cp_w1, cp_b1, cp_w2, cp_b2, cp_w3, cp_b3,
                w_w1, w_b1, w_w2, w_b2, w_w3, w_b3):
    wr = np.zeros((128, C_R), np.float32)
    wr[:, _C_W1T:_C_W1T + 128] = cp_w1.T       # [128,128]
    wr[:, _C_W2T:_C_W2T + 256] = cp_w2.T       # [128,256]
    w3t = cp_w3.T                              # [256,20]
    wr[:, _C_W3T:_C_W3T + 20] = w3t[0:128]
    wr[:, _C_W3T + 20:_C_W3T + 40] = w3t[128:256]
    wr[:, _C_WW1T:_C_WW1T + 64] = w_w1.T       # [128,64]
    wr[0:64, _C_WW2T:_C_WW2T + 128] = w_w2.T   # [64,128]
    w3w = w_w3.T                               # [128,10]
    wr[:, _C_WW3D + 0:_C_WW3D + 20:2] = w3w
    wr[:, _C_WW3D + 1:_C_WW3D + 20:2] = w3w
    wr[0:20, _C_BTSP:_C_BTSP + S] = sample_basis()
    wr = round_f32r(wr)

    wf = np.zeros((128, C_F), np.float32)
    wf[:, _C_B1] = cp_b1
    wf[:, _C_B2A] = cp_b2[0:128]
    wf[:, _C_B2B] = cp_b2[128:256]
    wf[0:20, _C_B3] = cp_b3
    wf[0:64, _C_WB1] = w_b1
    wf[:, _C_WB2] = w_b2
    wf[0:20, _C_WB3D:_C_WB3D + 1] = np.repeat(w_b3, 2)[:, None]
    im = interp_matrix()
    return wr, wf, im


_NC_CACHE = None


def get_program():
    global _NC_CACHE
    if _NC_CACHE is None:
        _NC_CACHE = build_program()
    return _NC_CACHE


def kernel(x, cp_w1, cp_b1, cp_w2, cp_b2, cp_w3, cp_b3,
           w_w1, w_b1, w_w2, w_b2, w_w3, w_b3, _return_raw=False):
    x = np.asarray(x, np.float32)
    wr, wf, im = host_consts(
        np.asarray(cp_w1, np.float32), np.asarray(cp_b1, np.float32),
        np.asarray(cp_w2, np.float32), np.asarray(cp_b2, np.float32),
        np.asarray(cp_w3, np.float32), np.asarray(cp_b3, np.float32),
        np.asarray(w_w1, np.float32), np.asarray(w_b1, np.float32),
        np.asarray(w_w2, np.float32), np.asarray(w_b2, np.float32),
        np.asarray(w_w3, np.float32), np.asarray(w_b3, np.float32))

    nc = get_program()
    in_maps = [
        {"x": np.ascontiguousarray(x[i * BC:(i + 1) * BC].T),
         "wr": wr, "wf": wf, "im": im}
        for i in range(NCORES)
    ]
    res = run_bass_kernel_spmd(nc, in_maps, list(range(NCORES)))
    outs = [res.results[i]["out"] for i in range(NCORES)]
    full = np.concatenate(outs, axis=0).astype(np.float32)
    if _return_raw:
        return full, res
    return full
